# revision 8
# baseline (speedup 1.0000x reference)
"""DeepAR LSTM kernel for Trainium2 (Bass/Tile), 8-core data parallel.

Layout (per core, B=1024 split into 2 batch-chunks of FD=512):
  R tile [105, 1024] (SBUF, ping-pong):
      rows 0-3  : u_t = [z_scaled; xc^T(3)]
      row  4    : ones (bias row, gate matmul)
      rows 5-63 : zeros
      rows 64-103: h_t
      row  104  : ones (bias row, mu/sigma matmul)
  Gate matmuls (K=105, M=128, N=512), weight cols:
      W_A: [2*g (0-39) | 0 | f (64-103) | 0]   (g scaled: sigmoid(2g+2b))
      W_B: [i (0-39)   | 0 | o (64-103) | 0]
  One sigmoid over PSUM [128, 1024] per chunk:
      S: g2 = S[0:40,0:512], f = S[64:104,0:512],
         i  = S[0:40,512:1024], o = S[64:104,512:1024]
  tanh(g) = 2*(sigmoid(2g) - 0.5):
      A2 = (S_g2 - 0.5) * S_i        (DVE STT)  -> AB[0:40]
      Bt = S_f * c_prev(PSUM)        (DVE TT)   -> AB[64:104]
      c  = [2I;0;I]^T @ AB           (PE)       -> c PSUM [40, 512] in place
      Tc = tanh(c)                   (ACT)      -> base-64 SBUF
      h  = Tc * S_o                  (DVE TT)   -> R_next[64:104]
  Prediction adds mu/sigma matmuls (h as stationary op, batch-major out),
  softplus via even polynomial on [128, 8] tiles, PE transposes to put
  y/v2 back into R's z-row.

Hardware constraints honored throughout:
  - every instruction carries at most ONE semaphore wait -> "touch" ops
    advance each engine's clock one dependency at a time
  - compute-op partition bases must be 32-aligned; two SBUF tensor operands
    must share a base partition (mixed SBUF/PSUM may differ)
  - DVE output window must not straddle partition 63/64
"""

import numpy as np

# ---------------------------------------------------------------- constants
B_TOT, CR, PR = 8192, 168, 24
E_SZ, I_SZ, H = 20, 3, 40
NCORES = 8
B = B_TOT // NCORES          # 1024
T_COND = CR - 1              # 167
FD = 512
NCH = B // FD                # 2
RK = 105                     # R rows (K of gate matmul)
NJ = B // 128                # 8 batch-major column groups

# softplus(s) = s/2 + Q(s*s)
_POLY_DEG = 8
_POLY_RANGE = 36.0           # u = s^2, |s| <= 6


def _fit_softplus_poly():
    u = np.linspace(0, _POLY_RANGE, 4001, dtype=np.float64)
    x = np.sqrt(u)
    target = np.logaddexp(0.0, x) - 0.5 * x
    cf = np.polynomial.chebyshev.chebfit(u, target, _POLY_DEG)
    q = np.polynomial.chebyshev.cheb2poly(cf)
    err = np.abs(np.polynomial.polynomial.polyval(u, q) - target).max()
    return q.astype(np.float64), float(err)


_Q, _Q_ERR = _fit_softplus_poly()


def _build_program():
    import concourse.bass as bass
    import concourse.tile as tile
    import concourse.mybir as mybir

    AF = mybir.ActivationFunctionType
    OP = mybir.AluOpType
    F32 = mybir.dt.float32

    nc = bass.Bass()

    wa_d = nc.declare_dram_parameter("wa", [RK, 128], F32, isOutput=False)
    wb_d = nc.declare_dram_parameter("wb", [RK, 128], F32, isOutput=False)
    wms_d = nc.declare_dram_parameter("wms", [RK, 2], F32, isOutput=False)
    idt_d = nc.declare_dram_parameter("idt", [128, 128], F32, isOutput=False)
    cid_d = nc.declare_dram_parameter("cid", [104, 40], F32, isOutput=False)
    ucond_d = nc.declare_dram_parameter("ucond", [T_COND, 4, B], F32, isOutput=False)
    upred_d = nc.declare_dram_parameter("upred", [PR, 3, B], F32, isOutput=False)
    zrow0_d = nc.declare_dram_parameter("zrow0", [1, B], F32, isOutput=False)
    ebm_d = nc.declare_dram_parameter("ebm", [128, PR * NJ], F32, isOutput=False)
    v2_d = nc.declare_dram_parameter("v2bm", [128, NJ], F32, isOutput=False)
    v2i_d = nc.declare_dram_parameter("v2ibm", [128, NJ], F32, isOutput=False)

    outy_d = nc.declare_dram_parameter("outy", [128, PR * NJ], F32, isOutput=True)
    outmu_d = nc.declare_dram_parameter("outmu", [128, PR * NJ], F32, isOutput=True)
    outsig_d = nc.declare_dram_parameter("outsig", [128, PR * NJ], F32, isOutput=True)

    with tile.TileContext(nc) as tc:
        with (
            tc.tile_pool(name="const", bufs=1) as cpool,
            tc.tile_pool(name="spool", bufs=2) as spool,
            tc.tile_pool(name="abpool", bufs=2) as abpool,
            tc.tile_pool(name="tcpool", bufs=2) as tcpool,
            tc.tile_pool(name="smallpool", bufs=3) as smallpool,
            tc.tile_pool(name="ppool", bufs=2, space="PSUM") as ppool,
            tc.tile_pool(name="cpsum", bufs=1, space="PSUM") as cpsum,
        ):
            # ---------------------------------------------- constants
            wa_t = cpool.tile([RK, 128], F32)
            wb_t = cpool.tile([RK, 128], F32)
            wms_t = cpool.tile([RK, 2], F32)
            idt_t = cpool.tile([128, 128], F32)
            cid_t = cpool.tile([104, 40], F32)
            ebm_t = cpool.tile([128, PR * NJ], F32)
            v2_t = cpool.tile([128, NJ], F32)
            v2i_t = cpool.tile([128, NJ], F32)
            nc.sync.dma_start(wa_t[:], wa_d[:])
            nc.sync.dma_start(wb_t[:], wb_d[:])
            nc.sync.dma_start(wms_t[:], wms_d[:])
            nc.sync.dma_start(idt_t[:], idt_d[:])
            nc.sync.dma_start(cid_t[:], cid_d[:])
            nc.sync.dma_start(ebm_t[:], ebm_d[:])
            nc.sync.dma_start(v2_t[:], v2_d[:])
            nc.sync.dma_start(v2i_t[:], v2i_d[:])

            r_t = [cpool.tile([RK, B], F32, tag=f"R{k}", name=f"R{k}") for k in range(2)]
            # c state: one PSUM tile per batch-chunk, updated in place
            c_ps = [cpsum.tile([40, FD], F32, tag=f"C{c}", name=f"C{c}") for c in range(NCH)]
            # transposed-y rows (PE transpose output must be PSUM partition 0)
            yr0 = cpsum.tile([1, FD], F32, tag="YR0")
            yr1 = cpsum.tile([1, FD], F32, tag="YR1")

            oy_t = cpool.tile([128, PR * NJ], F32)
            om_t = cpool.tile([128, PR * NJ], F32)
            os_t = cpool.tile([128, PR * NJ], F32)

            dj_t = cpool.tile([1, 8], F32)   # DVE junk

            # ---- init (all state-writes on DVE so consumers take 1 wait) ----
            for k in range(2):
                # partition bases must be 32-aligned: build the ones rows
                # (4 and 104) with aligned over-writes
                nc.vector.memset(r_t[k][:], 0.0)
                nc.vector.memset(r_t[k][0:5, :], 1.0)    # rows 0-3 re-DMA'd
                nc.vector.memset(r_t[k][96:105, :], 1.0)
                nc.vector.memset(r_t[k][96:104, :], 0.0)
            for c in range(NCH):
                nc.vector.memset(c_ps[c][:], 0.0)
            nc.vector.memset(oy_t[:], 0.0)
            nc.vector.memset(om_t[:], 0.0)
            nc.vector.memset(os_t[:], 0.0)
            # zero the AB slots once: rows 40-63 are never written afterwards
            ab_init = [abpool.tile([104, FD], F32, tag="AB", name=f"abz{_}") for _ in range(2)]
            for abz in ab_init:
                nc.vector.memset(abz[:], 0.0)

            # prime DVE clock on const DMA lanes
            nc.vector.tensor_copy(dj_t[0:1, 0:1], ebm_t[0:1, 0:1])
            nc.vector.tensor_copy(dj_t[0:1, 1:2], v2_t[0:1, 0:1])
            nc.vector.tensor_copy(dj_t[0:1, 2:3], v2i_t[0:1, 0:1])

            # prime PE clock on weight DMA lanes (junk psum tile, released)
            junk_p = ppool.tile([1, 8], F32, tag="P")
            nc.tensor.matmul(junk_p[0:1, 0:1], wa_t[0:1, 0:1], wa_t[0:1, 0:1],
                             start=True, stop=True)
            nc.tensor.matmul(junk_p[0:1, 1:2], wb_t[0:1, 0:1], wb_t[0:1, 0:1],
                             start=True, stop=True)
            nc.tensor.matmul(junk_p[0:1, 2:3], wms_t[0:1, 0:1], wms_t[0:1, 0:1],
                             start=True, stop=True)
            nc.tensor.matmul(junk_p[0:1, 3:4], idt_t[0:1, 0:1], idt_t[0:1, 0:1],
                             start=True, stop=True)
            nc.tensor.matmul(junk_p[0:1, 4:5], cid_t[0:1, 0:1], cid_t[0:1, 0:1],
                             start=True, stop=True)

            nc.sync.dma_start(r_t[0][0:4, :], ucond_d[0, :, :])

            # ------------------------------------------------- one LSTM step
            def lstm_step(t_glob, is_pred, pred_j):
                rc = r_t[t_glob % 2]
                rn = r_t[(t_glob + 1) % 2]

                # touchU0/touchU: absorb z-row writer then u-DMA lane.
                # They write a corner of P(c0) that MM_A overwrites.
                p_c = [ppool.tile([128, 2 * FD], F32, tag="P", name=f"P{c}")
                       for c in range(NCH)]
                nc.tensor.matmul(p_c[0][0:1, 0:1], rc[0:1, 0:1], rc[0:1, 0:1],
                                 start=True, stop=True)
                nc.tensor.matmul(p_c[0][0:1, 1:2], rc[0:5, 0:1], rc[0:5, 0:1],
                                 start=True, stop=True)
                for c in range(NCH):
                    cols = slice(c * FD, (c + 1) * FD)
                    nc.tensor.matmul(p_c[c][:, 0:FD], wa_t[:], rc[:, cols],
                                     start=True, stop=True)
                    nc.tensor.matmul(p_c[c][:, FD:2 * FD], wb_t[:], rc[:, cols],
                                     start=True, stop=True)
                for c in range(NCH):
                    s = spool.tile([128, 2 * FD], F32, tag="S")
                    # touchS: absorb this S slot's DVE WAR tick
                    nc.scalar.copy(s[0:1, 0:1], wa_t[0:1, 0:1])
                    nc.scalar.activation(s[:], p_c[c][:], AF.Sigmoid)

                    ab = abpool.tile([104, FD], F32, tag="AB")
                    # touchD: absorb PE tick (c matmul of t-1) on DVE
                    nc.vector.tensor_copy(dj_t[0:1, 3:4], c_ps[c][0:1, 0:1])
                    nc.vector.tensor_tensor(ab[64:104, :], s[64:104, 0:FD],
                                            c_ps[c][:], OP.mult)
                    nc.vector.scalar_tensor_tensor(ab[0:40, :], s[0:40, 0:FD],
                                                   0.5, s[0:40, FD:2 * FD],
                                                   OP.subtract, OP.mult)
                    # touchC (absorbs ACT tanh(t-1) WAR) then c update
                    nc.tensor.matmul(c_ps[c][0:1, 0:1], cid_t[0:1, 0:1],
                                     ab[0:1, 0:1], start=True, stop=True)
                    nc.tensor.matmul(c_ps[c][:], cid_t[:], ab[:],
                                     start=True, stop=True)
                    tcx = tcpool.tile([104, FD], F32, tag="TC")
                    nc.scalar.activation(tcx[64:104, :], c_ps[c][:], AF.Tanh)
                    nc.vector.tensor_tensor(rn[64:104, c * FD:(c + 1) * FD],
                                            tcx[64:104, :], s[64:104, FD:2 * FD],
                                            OP.mult)

                # prefetch u rows for next step
                t_next = t_glob + 1
                if t_next < T_COND:
                    nc.sync.dma_start(rn[0:4, :], ucond_d[t_next, :, :])
                elif t_next == T_COND:
                    nc.sync.dma_start(rn[1:4, :], upred_d[0, :, :])
                    nc.sync.dma_start(rn[0:1, :], zrow0_d[:])
                elif t_next < T_COND + PR:
                    nc.sync.dma_start(rn[1:4, :], upred_d[t_next - T_COND, :, :])

                if not is_pred:
                    return

                # ---------------- prediction extras ----------------
                j8 = pred_j * NJ
                msp = ppool.tile([128, 2 * NJ], F32, tag="P")
                # touchMS: absorb msp slot's WAR
                nc.tensor.matmul(msp[0:1, 0:1], wms_t[0:1, 0:1], wms_t[0:1, 0:1],
                                 start=True, stop=True)
                for j in range(NJ):
                    nc.tensor.matmul(msp[:, 2 * j:2 * j + 2],
                                     rn[64:105, j * 128:(j + 1) * 128],
                                     wms_t[64:105, :], start=True, stop=True)
                msb = smallpool.tile([128, 2 * NJ], F32, tag="MSB")
                nc.vector.tensor_copy(msb[:], msp[:])
                mu = msb[:, 0:2 * NJ:2]
                s_ = msb[:, 1:2 * NJ:2]

                w1 = smallpool.tile([128, NJ], F32, tag="W1")
                w2 = smallpool.tile([128, NJ], F32, tag="W2")
                sg = smallpool.tile([128, NJ], F32, tag="SG")
                yv = smallpool.tile([128, NJ], F32, tag="YV")
                yt = smallpool.tile([128, NJ], F32, tag="YT")
                nc.vector.tensor_tensor(w1[:], s_, s_, OP.mult)
                nc.vector.tensor_scalar_mul(w2[:], w1[:], float(_Q[_POLY_DEG]))
                for k in range(_POLY_DEG - 1, 0, -1):
                    nc.vector.scalar_tensor_tensor(w2[:], w2[:], float(_Q[k]),
                                                   w1[:], OP.add, OP.mult)
                nc.vector.tensor_scalar_add(w2[:], w2[:], float(_Q[0]))
                nc.vector.scalar_tensor_tensor(sg[:], s_, 0.5, w2[:],
                                               OP.mult, OP.add)
                nc.vector.tensor_tensor(w1[:], sg[:], ebm_t[:, j8:j8 + NJ], OP.mult)
                nc.vector.tensor_tensor(yv[:], w1[:], mu, OP.add)
                nc.vector.tensor_tensor(oy_t[:, j8:j8 + NJ], yv[:], v2_t[:], OP.mult)
                nc.vector.tensor_tensor(om_t[:, j8:j8 + NJ], mu, v2_t[:], OP.mult)
                nc.vector.tensor_tensor(os_t[:, j8:j8 + NJ], sg[:], v2_t[:], OP.mult)

                if pred_j == PR - 1:
                    return
                nc.vector.tensor_tensor(yt[:], yv[:], v2i_t[:], OP.mult)
                # touchY: absorb ACT zrow-copy(t-1) WAR on the yr tiles
                nc.tensor.matmul(yr0[0:1, 0:1], idt_t[0:1, 0:1], idt_t[0:1, 0:1],
                                 start=True, stop=True)
                nc.tensor.matmul(yr1[0:1, 0:1], idt_t[0:1, 0:1], idt_t[0:1, 0:1],
                                 start=True, stop=True)
                for j in range(NJ):
                    yr = yr0 if j < 4 else yr1
                    colo = (j % 4) * 128
                    nc.tensor.transpose(yr[0:1, colo:colo + 128],
                                        yt[:, j:j + 1], idt_t[:])
                nc.scalar.copy(rn[0:1, 0:FD], yr0[0:1, :])
                nc.scalar.copy(rn[0:1, FD:2 * FD], yr1[0:1, :])

            for t in range(T_COND):
                lstm_step(t, False, -1)
            for jp in range(PR):
                lstm_step(T_COND + jp, True, jp)

            nc.sync.dma_start(outy_d[:], oy_t[:])
            nc.sync.dma_start(outmu_d[:], om_t[:])
            nc.sync.dma_start(outsig_d[:], os_t[:])

    _split_multi_waits(nc, mybir)
    return nc


def _split_multi_waits(nc, mybir):
    """Each TPB instruction has exactly one sem-wait slot. Tile sometimes
    emits several waits on one instruction; carry the extras on same-engine
    NoOps inserted right before it."""
    n_split = 0
    for f in nc.m.functions:
        for blk in f.blocks:
            new = []
            for inst in blk.instructions:
                si = inst.sync_info
                waits = list(si.on_wait) if si is not None else []
                if len(waits) > 1:
                    n_split += 1
                    for w in waits[:-1]:
                        nop = mybir.InstNoOp(
                            name=nc.get_next_instruction_name(), ins=[], outs=[])
                        nop.engine = inst.engine
                        nop.sync_info = mybir.SyncInfo(on_wait=[w], on_update=[])
                        new.append(nop)
                    inst.sync_info = mybir.SyncInfo(
                        on_wait=[waits[-1]], on_update=list(si.on_update))
                new.append(inst)
            blk.instructions = new
    return n_split


def _prep_weights(W_emb, b_emb, W_ih, b_ih, W_hh, b_hh, W_mu, b_mu, W_sig, b_sig):
    W_emb = W_emb.astype(np.float64)
    W_ih = W_ih.astype(np.float64)
    W_hh = W_hh.astype(np.float64)
    W_z = (W_ih[:, :E_SZ] @ W_emb)[:, 0]
    W_x = W_ih[:, E_SZ:E_SZ + I_SZ]
    b_eff = (W_ih[:, :E_SZ] @ b_emb.astype(np.float64)
             + b_ih.astype(np.float64) + b_hh.astype(np.float64))

    def gate(g):
        return slice(g * H, (g + 1) * H)
    idx_i, idx_f, idx_g, idx_o = gate(0), gate(1), gate(2), gate(3)

    def fill(w, cols, rows, scale):
        w[0:1, cols] = W_z[rows][None, :] * scale
        w[1:4, cols] = W_x[rows].T * scale
        w[4:5, cols] = b_eff[rows][None, :] * scale
        w[64:104, cols] = W_hh[rows].T * scale

    wa = np.zeros((RK, 128), np.float64)
    wb = np.zeros((RK, 128), np.float64)
    fill(wa, slice(0, 40), idx_g, 2.0)
    fill(wa, slice(64, 104), idx_f, 1.0)
    fill(wb, slice(0, 40), idx_i, 1.0)
    fill(wb, slice(64, 104), idx_o, 1.0)

    wms = np.zeros((RK, 2), np.float64)
    wms[64:104, 0] = W_mu[0].astype(np.float64)
    wms[64:104, 1] = W_sig[0].astype(np.float64)
    wms[104, 0] = float(np.asarray(b_mu).reshape(-1)[0])
    wms[104, 1] = float(np.asarray(b_sig).reshape(-1)[0])

    cid = np.zeros((104, 40), np.float64)
    cid[0:40, :] = 2.0 * np.eye(40)
    cid[64:104, :] = np.eye(40)

    return (wa.astype(np.float32), wb.astype(np.float32),
            wms.astype(np.float32), cid.astype(np.float32))


_PROGRAM_CACHE = {}


def kernel(z1, xc, W_emb, b_emb, W_ih, b_ih, W_hh, b_hh, W_mu, b_mu, W_sig, b_sig,
           conditioning_range=168, prediction_range=24):
    from concourse.bass_utils import run_bass_kernel_spmd

    z1 = np.asarray(z1, np.float32)
    xc = np.asarray(xc, np.float32)
    assert int(conditioning_range) == CR and int(prediction_range) == PR
    assert z1.shape == (B_TOT, CR, 1) and xc.shape == (B_TOT, CR + PR, I_SZ)

    wa, wb, wms, cid = _prep_weights(
        np.asarray(W_emb), np.asarray(b_emb), np.asarray(W_ih), np.asarray(b_ih),
        np.asarray(W_hh), np.asarray(b_hh), np.asarray(W_mu), np.asarray(b_mu),
        np.asarray(W_sig), np.asarray(b_sig))

    import jax
    import jax.numpy as jnp
    with jax.default_device(jax.devices("cpu")[0]):
        eps = np.asarray(jax.random.normal(jax.random.key(42), (PR, B_TOT, 1),
                                           jnp.float32))

    idt = np.eye(128, dtype=np.float32)
    v_all = (np.mean(z1[:, :, 0], axis=1, dtype=np.float32) + 1.0).astype(np.float32)

    in_maps = []
    for core in range(NCORES):
        sl = slice(core * B, (core + 1) * B)
        z1c = z1[sl, :, 0]
        xcc = xc[sl]
        v = v_all[sl]
        zs = (z1c[:, :T_COND] / v[:, None]).astype(np.float32)

        ucond = np.empty((T_COND, 4, B), np.float32)
        ucond[:, 0, :] = zs.T
        ucond[:, 1:4, :] = xcc[:, 1:CR, :].transpose(1, 2, 0)

        upred = np.ascontiguousarray(
            xcc[:, CR:CR + PR, :].transpose(1, 2, 0)).astype(np.float32)
        zrow0 = (z1c[:, CR - 1] / v).astype(np.float32)[None, :]

        epsc = eps[:, sl, 0]
        ebm = np.ascontiguousarray(
            epsc.reshape(PR, NJ, 128).transpose(2, 0, 1).reshape(128, PR * NJ)
        ).astype(np.float32)
        v2bm = np.ascontiguousarray(v.reshape(NJ, 128).T).astype(np.float32)
        v2ibm = (1.0 / v2bm).astype(np.float32)

        in_maps.append({
            "wa": wa, "wb": wb, "wms": wms, "idt": idt, "cid": cid,
            "ucond": ucond, "upred": upred, "zrow0": zrow0,
            "ebm": ebm, "v2bm": v2bm, "v2ibm": v2ibm,
        })

    if "nc" not in _PROGRAM_CACHE:
        _PROGRAM_CACHE["nc"] = _build_program()
    nc = _PROGRAM_CACHE["nc"]

    _res = run_bass_kernel_spmd(nc, in_maps, list(range(NCORES)))
    _PROGRAM_CACHE["last_results"] = _res
    results = _res.results

    def unstage(name):
        full = np.empty((B_TOT, PR), np.float32)
        for core in range(NCORES):
            st = results[core][name].reshape(128, PR, NJ)
            full[core * B:(core + 1) * B] = st.transpose(2, 0, 1).reshape(B, PR)
        return full[:, :, None]

    return unstage("outy"), unstage("outmu"), unstage("outsig")
